# revision 3
# baseline (speedup 1.0000x reference)
"""Trainium2 Bass kernel for nn_D_GCN, v2 (c2r2 sharding).

Reference (per batch b):
    w   = h1 + A h2          (pass 1, all N rows)
    out = relu(g0 + A w)     (pass 2)
with g0/h1/h2 host-precomputed feature-space projections (Theta folded).

Sharding: 8 cores = 2 batches x 2 column-halves (TO=256 -> 128) x 2
row-halves (N=4096 -> 2048). Each core:
  pass 1: w[all 4096 rows, my 128 cols]   (2x redundant per batch, not 4x)
  pass 2: out[my 2048 rows, my 128 cols]
No collectives (first NRT collective costs ~130us on this runtime).

A^T is fully SBUF-resident (16 MiB fp8), streamed as 32 column-stripes
of 0.5 MiB. Slot order: node-row chunks are permuted per core so the
SPMD program is uniform: slots 0..15 = my row chunks (pass-2 lhsT),
slots 16..31 = the other half. Pass 1 processes pool slots (16..31)
first while my stripes stream, then mine; pass 2's k-step kappa
consumes w slot-pair kappa as soon as pass 1 produces it, interleaved.

Scales: A x4096 (fp8), w x16 (fp8), exact f32 g0 carries the dominant
term (same numerics as the 97us baseline, rel err ~6e-4).
"""

import sys

if "/opt/trn_rl_repo" not in sys.path:
    sys.path.insert(0, "/opt/trn_rl_repo")

import numpy as np
import ml_dtypes

B, T, N, F, O = 2, 8, 4096, 32, 32
K = 3
TO = T * O          # 256
NCORES = 8
NCH = 32            # node chunks of 128 rows
RCH = 16            # my row chunks (pass 2)
MC = 128            # my columns
KP = NCH // 2       # 16 DR k-pairs

SCALE_A = 4096.0
SCALE_W = 16.0

_CACHE = {}


def _build_nc():
    import concourse.mybir as mybir
    import concourse.tile as tile
    from concourse import bacc

    f32 = mybir.dt.float32
    bf16 = mybir.dt.bfloat16
    fp8 = mybir.dt.float8e4
    DR = mybir.MatmulPerfMode.DoubleRow
    Relu = mybir.ActivationFunctionType.Relu

    nc = bacc.Bacc(None, num_devices=NCORES)

    # stripe s: A^T[slot-ordered k, rows of slot s] as [128, 32, 128]
    AST_d = nc.dram_tensor("AST", [NCH, 128, NCH, MC], fp8,
                           kind="ExternalInput")
    H2_d = nc.dram_tensor("H2", [128, NCH, MC], fp8, kind="ExternalInput")
    H1P_d = nc.dram_tensor("H1P", [128, RCH, MC], bf16, kind="ExternalInput")
    H1R_d = nc.dram_tensor("H1R", [128, RCH, MC], bf16, kind="ExternalInput")
    G0_d = nc.dram_tensor("G0", [128, RCH, MC], bf16, kind="ExternalInput")
    OUT_d = nc.dram_tensor("OUT", [128, RCH, MC], bf16, kind="ExternalOutput")

    with tile.TileContext(nc) as tc:
        with (
            tc.tile_pool(name="big", bufs=1) as big,
            tc.tile_pool(name="ps", bufs=1, space="PSUM") as psp,
        ):
            ST = [big.tile([128, NCH, MC], fp8, name=f"st{s}", tag=f"st{s}")
                  for s in range(NCH)]
            H2 = big.tile([128, NCH, MC], fp8, name="H2s", tag="H2s")
            H1P = big.tile([128, RCH, MC], bf16, name="H1Ps", tag="H1Ps")
            H1R = big.tile([128, RCH, MC], bf16, name="H1Rs", tag="H1Rs")
            G0 = big.tile([128, RCH, MC], bf16, name="G0s", tag="G0s")
            # w slot-pair j (slots 2j, 2j+1), fp8 x16
            WSp = [big.tile([128, 2, MC], fp8, name=f"ws{j}", tag=f"ws{j}")
                   for j in range(KP)]
            # output in 4 groups of 4 chunks (separate tiles so a group's
            # DMA never WAR-couples to later chunks' writes)
            OSg = [big.tile([128, 4, MC], bf16, name=f"osg{g}", tag=f"osg{g}")
                   for g in range(4)]
            # pass-2 round-1 partials: (pool-w psum)/65536 + g0
            P1s = [big.tile([128, MC], f32, name=f"p1s{v}", tag=f"p1s{v}")
                   for v in range(RCH)]

            # ---- sync ring: H2 then a PURE stripe stream in consumption
            # order (no mid-stream inserts to starve the PE). The small
            # non-A tensors go on the otherwise-idle scalar ring, issued up
            # front: their engine contention lands in the warm-up window,
            # and their consumers (STTs on DVE) tolerate lag.
            nc.scalar.dma_start(H1P[:], H1P_d[:])
            nc.scalar.dma_start(H1R[:], H1R_d[:])
            nc.scalar.dma_start(G0[:], G0_d[:])
            nc.sync.dma_start(H2[:], H2_d[:])
            for s in range(16, NCH):
                nc.sync.dma_start(ST[s][:], AST_d[s])
            for s in range(RCH):
                nc.sync.dma_start(ST[s][:], AST_d[s])

            # ---- PE warm-up (clock ramp) over a zeroed tile ----
            warm_src = big.tile([128, 2, 256], fp8, name="warmsrc",
                                tag="warmsrc")
            nc.gpsimd.memset(warm_src[:], 0.0)
            warm_ps = psp.tile([128, 256], f32, name="warm", tag="warm")
            for wi in range(40):
                nc.tensor.matmul(
                    warm_ps[:], warm_src[:, :, 0:128], warm_src[:],
                    start=(wi == 0), stop=(wi == 39), perf_mode=DR)

            # PSUM: 8 banks, bank-granular. warm 1 + p1 rotation 4 +
            # p2 rotation 2 = 7.
            p1ps = [psp.tile([128, MC], f32, name=f"w{i}", tag=f"w{i}")
                    for i in range(4)]
            p2ps = [psp.tile([128, MC], f32, name=f"o{i}", tag=f"o{i}")
                    for i in range(3)]

            def p1_chunk(s):
                """w slot s = (A^T stripe_s)^T h2 -> WSp, fp8 x16."""
                pst = p1ps[s % 4]
                for kp in range(KP):
                    nc.tensor.matmul(
                        pst[:], ST[s][:, 2 * kp:2 * kp + 2, :],
                        H2[:, 2 * kp:2 * kp + 2, :],
                        start=(kp == 0), stop=(kp == KP - 1), perf_mode=DR)
                h1 = H1P if s >= RCH else H1R
                hs = s - RCH if s >= RCH else s
                nc.vector.scalar_tensor_tensor(
                    WSp[s // 2][:, s % 2, :], pst[:], 1.0 / 256.0,
                    h1[:, hs, :], mybir.AluOpType.mult, mybir.AluOpType.add)

            def p2_round1(v):
                """out chunk v: pool-w half of the contraction; spill
                scaled + g0 so round 2's epilogue is one STT + one max."""
                pst = p2ps[v % 3]
                for kap in range(RCH // 2, KP):
                    nc.tensor.matmul(
                        pst[:], ST[v][:, 2 * kap:2 * kap + 2, :],
                        WSp[kap][:],
                        start=(kap == RCH // 2), stop=(kap == KP - 1),
                        perf_mode=DR)
                nc.vector.scalar_tensor_tensor(
                    P1s[v][:], pst[:], 1.0 / 65536.0, G0[:, v, :],
                    mybir.AluOpType.mult, mybir.AluOpType.add)

            def p2_round2(v):
                """out chunk v: my-w half, combine + relu on DVE, bf16."""
                pst = p2ps[v % 3]
                for kap in range(RCH // 2):
                    nc.tensor.matmul(
                        pst[:], ST[v][:, 2 * kap:2 * kap + 2, :],
                        WSp[kap][:],
                        start=(kap == 0), stop=(kap == RCH // 2 - 1),
                        perf_mode=DR)
                o = OSg[v // 4][:, v % 4, :]
                nc.vector.scalar_tensor_tensor(
                    o, pst[:], 1.0 / 65536.0, P1s[v][:],
                    mybir.AluOpType.mult, mybir.AluOpType.add)
                nc.vector.tensor_scalar_max(o, o, 0.0)

            with nc.named_scope("pool"):
                # pool slots 16..31: pass 1 only (my stripes not in yet)
                for s in range(RCH, NCH):
                    p1_chunk(s)

            with nc.named_scope("res"):
                # my slots 0..15: p1 chunk pair, then p2 round 1 for the
                # two freshly-arrived stripes (pool w is complete)
                for i in range(8):
                    p1_chunk(2 * i)
                    p1_chunk(2 * i + 1)
                    p2_round1(2 * i)
                    p2_round1(2 * i + 1)

            with nc.named_scope("r2"):
                for v in range(RCH):
                    p2_round2(v)
                    if v % 4 == 3:
                        g = v // 4
                        nc.scalar.dma_start(OUT_d[:, 4 * g:4 * g + 4, :],
                                            OSg[g][:])

    nc.compile()
    return nc


def _get_nc():
    if "nc" not in _CACHE:
        _CACHE["nc"] = _build_nc()
    return _CACHE["nc"]


def _prepare_in_maps(X, A_q, Theta1, bias):
    fp8 = ml_dtypes.float8_e4m3
    bf16 = ml_dtypes.bfloat16
    X = np.asarray(X, dtype=np.float32)
    A_q = np.asarray(A_q, dtype=np.float32)
    Theta1 = np.asarray(Theta1, dtype=np.float32)
    bias = np.asarray(bias, dtype=np.float32)

    Th = Theta1.reshape(F, K, O)
    Th0, Th1, Th2 = Th[:, 0], Th[:, 1], Th[:, 2]

    in_maps = [None] * NCORES
    for b in range(B):
        Xb = X[b]                                   # (T, N, F)
        h2 = np.transpose(2.0 * (Xb @ Th2), (1, 0, 2)).reshape(N, TO)
        h1 = np.transpose(Xb @ Th1, (1, 0, 2)).reshape(N, TO)
        g0 = np.transpose(Xb @ (Th0 - Th2) + bias, (1, 0, 2)).reshape(N, TO)
        AT8 = (A_q[b].T * SCALE_A).astype(fp8)      # [m, n]
        ATc = AT8.reshape(NCH, 128, NCH, 128)       # [mc, p, ncn, j]
        for rq in range(2):
            # slot order: my 16 chunks first, then the other 16
            perm = np.r_[np.arange(rq * 16, rq * 16 + 16),
                         np.arange((1 - rq) * 16, (1 - rq) * 16 + 16)]
            # AST[s] = [p, s_k, j] = ATc[perm[s_k], p, perm[s], :]
            AST = np.ascontiguousarray(
                ATc[perm][:, :, perm, :].transpose(2, 1, 0, 3))
            h2s = h2[perm.repeat(128) * 128 +
                     np.tile(np.arange(128), NCH)]   # rows in slot order
            for cq in range(2):
                myc = slice(cq * MC, (cq + 1) * MC)
                H2c = np.ascontiguousarray(
                    h2s[:, myc].reshape(NCH, 128, MC)
                    .transpose(1, 0, 2)).astype(fp8)
                h1p = np.ascontiguousarray(
                    (SCALE_W * h1[perm[16:].repeat(128) * 128 +
                                  np.tile(np.arange(128), RCH)][:, myc])
                    .reshape(RCH, 128, MC).transpose(1, 0, 2)).astype(bf16)
                h1r = np.ascontiguousarray(
                    (SCALE_W * h1[perm[:16].repeat(128) * 128 +
                                  np.tile(np.arange(128), RCH)][:, myc])
                    .reshape(RCH, 128, MC).transpose(1, 0, 2)).astype(bf16)
                g0r = np.ascontiguousarray(
                    g0[perm[:16].repeat(128) * 128 +
                       np.tile(np.arange(128), RCH)][:, myc]
                    .reshape(RCH, 128, MC).transpose(1, 0, 2)).astype(bf16)
                core = b * 4 + rq * 2 + cq
                in_maps[core] = {
                    "AST": AST,
                    "H2": H2c,
                    "H1P": h1p,
                    "H1R": h1r,
                    "G0": g0r,
                }
    return in_maps


def run_with_results(inputs, **spmd_kwargs):
    from concourse.bass_utils import run_bass_kernel_spmd

    nc = _get_nc()
    in_maps = _prepare_in_maps(**inputs)
    res = run_bass_kernel_spmd(
        nc, in_maps, core_ids=list(range(NCORES)), **spmd_kwargs)

    out = np.empty((B, T, N, O), dtype=np.float32)
    for c in range(NCORES):
        b, rq, cq = c // 4, (c % 4) // 2, c % 2
        blk = np.asarray(res.results[c]["OUT"],
                         dtype=np.float32)       # [128, RCH, MC]
        rows = np.transpose(blk, (1, 0, 2)).reshape(RCH * 128, T // 2, O)
        tsl = slice(cq * 4, cq * 4 + 4)
        nsl = slice(rq * 2048, (rq + 1) * 2048)
        out[b, tsl, nsl, :] = np.transpose(rows, (1, 0, 2))
    return out, res


def kernel(X, A_q, Theta1, bias):
    out, _ = run_with_results(
        {"X": X, "A_q": A_q, "Theta1": Theta1, "bias": bias})
    return out
